# revision 1
# baseline (speedup 1.0000x reference)
"""TRN2 Bass kernel for nn_Construct_76484777607483.

Computes, for 12 input tensors x_i [B=2, C=256, H=64, W=256]:
    y_i = einsum('bchw,co->bohw', x_i, W)
interleaved over H (output row 12*h + i comes from tensor i, row h) into
out [2, 256, 768, 256], plus bias b[o] * count(row) where count is the
conv-transpose overlap multiplicity (ramp 1..12 at the top edge, 12 in the
middle, 12..1 at the bottom edge).

Sharding: 8 cores = (2 batches) x (4 h-quarters of 16 input rows). Each core
handles all 12 tensors for its 16 rows, so the row-interleave is assembled
on-chip and output DMA writes are fully contiguous per channel.

Per-core kernel: for each group of 2 input rows (512 pixels), for each tensor
i, a [256 -> 256] channel matmul is done as 2 accumulating 128x128x512
matmuls in float32r (full-rate PE path, ~1.5e-4 rel err), then the PSUM tile
is copied into an interleave-layout SBUF buffer with the per-(i, h) bias
value added as a per-partition scalar (DVE tensor_scalar_add). The bias
values (b[o] * count) are precomputed on host per core.
"""

import numpy as np

import concourse.bacc as bacc
import concourse.tile as tile
import concourse.mybir as mybir
from concourse.bass_utils import run_bass_kernel_spmd

B, C, H, WD = 2, 256, 64, 256
NT = 12                 # stacked tensors
NCORES = 8
HQ = H // 4             # 16 input rows per core
NG = HQ // 2            # 8 groups of 2 rows
HOUT = NT * H           # 768

_F32 = mybir.dt.float32
_F32R = mybir.dt.float32r

_NC_CACHE = {}


def build_nc():
    if "nc" in _NC_CACHE:
        return _NC_CACHE["nc"]
    nc = bacc.Bacc("TRN2", target_bir_lowering=False)
    x_d = nc.declare_dram_parameter("x", [NT, C, HQ, WD], _F32R, isOutput=False)
    w_d = nc.declare_dram_parameter("w", [C, C], _F32R, isOutput=False)
    bv_d = nc.declare_dram_parameter("bv", [2, 128, NT * HQ], _F32, isOutput=False)
    y_d = nc.declare_dram_parameter("y", [C, NT * HQ, WD], _F32, isOutput=True)

    with tile.TileContext(nc) as tc:
        with (
            tc.tile_pool(name="const", bufs=1) as cpool,
            tc.tile_pool(name="xin", bufs=6) as inpool,
            tc.tile_pool(name="obuf", bufs=3) as outpool,
            tc.tile_pool(name="ps", bufs=4, space="PSUM") as pspool,
        ):
            wt = [
                [
                    cpool.tile([128, 128], _F32R, name=f"w{kh}{mh}")
                    for mh in range(2)
                ]
                for kh in range(2)
            ]
            for kh in range(2):
                for mh in range(2):
                    # consts load on the ACT ring so the SP ring starts the
                    # first input tiles immediately
                    nc.scalar.dma_start(
                        out=wt[kh][mh][:],
                        in_=w_d[kh * 128 : (kh + 1) * 128, mh * 128 : (mh + 1) * 128],
                    )
            bvt = [cpool.tile([128, NT * HQ], _F32, name=f"bv{mh}") for mh in range(2)]
            for mh in range(2):
                nc.scalar.dma_start(out=bvt[mh][:], in_=bv_d[mh])

            for g in range(NG):
                obufs = [
                    outpool.tile(
                        [128, 2, NT, WD], _F32, name=f"ob{g}_{mh}", tag=f"ob{mh}"
                    )
                    for mh in range(2)
                ]
                for i0 in range(0, NT, 2):
                    xps = []
                    for i in (i0, i0 + 1):
                        xin = inpool.tile(
                            [128, 2, 2, WD], _F32R, name=f"xin{g}_{i}", tag="xin"
                        )
                        for kh in range(2):
                            eng = nc.gpsimd if (kh == 1 and i >= 5) else nc.sync
                            eng.dma_start(
                                out=xin[:, kh],
                                in_=x_d[
                                    i, kh * 128 : (kh + 1) * 128, 2 * g : 2 * g + 2, :
                                ],
                            )
                        xps.append(xin)
                    for mh in range(2):
                        # one 2-bank PSUM tile per tensor PAIR [128, ip, hl, WD]
                        ps = pspool.tile(
                            [128, 2, 2, WD], _F32, name=f"ps{g}_{i0}_{mh}", tag="ps"
                        )
                        for ip in range(2):
                            nc.tensor.matmul(
                                ps[:, ip],
                                wt[0][mh][:],
                                xps[ip][:, 0],
                                start=True,
                                stop=False,
                            )
                            nc.tensor.matmul(
                                ps[:, ip],
                                wt[1][mh][:],
                                xps[ip][:, 1],
                                start=False,
                                stop=True,
                            )
                        # ~1/6 of the PSUM->SBUF bias-add copies run on the
                        # ACT engine (activation Identity with per-partition
                        # bias), the rest on DVE, balancing both engines
                        on_act = i0 == 10
                        if g in (0, NG - 1):
                            # one of the two rows is the 0/63 boundary row,
                            # whose bias count varies per tensor: copy that
                            # row per tensor, merge the uniform row per pair
                            hv = 0 if g == 0 else 1  # varying-count row
                            hu = 1 - hv
                            for ip in range(2):
                                col = (i0 + ip) * HQ + 2 * g + hv
                                if on_act:
                                    nc.scalar.activation(
                                        obufs[mh][:, hv, i0 + ip],
                                        ps[:, ip, hv],
                                        mybir.ActivationFunctionType.Identity,
                                        bias=bvt[mh][:, col : col + 1],
                                    )
                                else:
                                    nc.vector.tensor_scalar_add(
                                        obufs[mh][:, hv, i0 + ip],
                                        ps[:, ip, hv],
                                        bvt[mh][:, col : col + 1],
                                    )
                            col = i0 * HQ + 2 * g + hu
                            if on_act:
                                nc.scalar.activation(
                                    obufs[mh][:, hu, i0 : i0 + 2],
                                    ps[:, :, hu],
                                    mybir.ActivationFunctionType.Identity,
                                    bias=bvt[mh][:, col : col + 1],
                                )
                            else:
                                nc.vector.tensor_scalar_add(
                                    obufs[mh][:, hu, i0 : i0 + 2],
                                    ps[:, :, hu],
                                    bvt[mh][:, col : col + 1],
                                )
                        else:
                            # interior rows: count uniform (12) across both
                            # tensors and rows -> one op per pair
                            col = i0 * HQ + 2 * g
                            src = ps[:].transpose([0, 2, 1, 3])  # (hl, ip, w)
                            if on_act:
                                nc.scalar.activation(
                                    obufs[mh][:, :, i0 : i0 + 2],
                                    src,
                                    mybir.ActivationFunctionType.Identity,
                                    bias=bvt[mh][:, col : col + 1],
                                )
                            else:
                                nc.vector.tensor_scalar_add(
                                    obufs[mh][:, :, i0 : i0 + 2],
                                    src,
                                    bvt[mh][:, col : col + 1],
                                )
                for mh in range(2):
                    # outputs split across the ACT HWDGE ring and the SWDGE
                    # (gpsimd) ring; small pieces keep each DMA-lane hold
                    # short to avoid head-of-line blocking
                    for q in range(6):
                        eng = (
                            nc.gpsimd
                            if (mh * 6 + q) in (1, 3, 5, 8, 10)
                            else nc.scalar
                        )
                        eng.dma_start(
                            out=y_d[
                                mh * 128 : (mh + 1) * 128,
                                24 * g + 4 * q : 24 * g + 4 * (q + 1),
                                :,
                            ],
                            in_=obufs[mh][:, q // 3, (q % 3) * 4 : (q % 3) * 4 + 4],
                        )
    nc.finalize()
    _NC_CACHE["nc"] = nc
    return nc


def _counts() -> np.ndarray:
    """count[r] for output row r (conv-transpose bias multiplicity)."""
    r = np.arange(HOUT)
    return (np.minimum(11, r) - np.maximum(0, r - (HOUT - NT)) + 1).astype(np.float32)


def shard_inputs(inputs: dict) -> list[dict]:
    xs = [np.ascontiguousarray(np.asarray(inputs[f"x{i}"], dtype=np.float32)) for i in range(NT)]
    w = np.ascontiguousarray(np.asarray(inputs["W"], dtype=np.float32))
    b = np.asarray(inputs["b"], dtype=np.float32)
    counts = _counts()
    in_maps = []
    for cid in range(NCORES):
        b_idx, hq = divmod(cid, 4)
        h0 = hq * HQ
        x_core = np.empty((NT, C, HQ, WD), dtype=np.float32)
        for i in range(NT):
            x_core[i] = xs[i][b_idx, :, h0 : h0 + HQ, :]
        # bv[mh, o, i*HQ + hl] = b[mh*128+o] * count(12*(h0+hl) + i)
        i_idx = np.arange(NT)[:, None]
        hl_idx = np.arange(HQ)[None, :]
        cnt = counts[12 * (h0 + hl_idx) + i_idx].reshape(NT * HQ)  # [192]
        bv = (b.reshape(2, 128)[:, :, None] * cnt[None, None, :]).astype(np.float32)
        in_maps.append({"x": x_core, "w": w, "bv": bv})
    return in_maps


def gather_outputs(results: list[dict]) -> np.ndarray:
    out = np.empty((B, C, HOUT, WD), dtype=np.float32)
    for cid in range(NCORES):
        b_idx, hq = divmod(cid, 4)
        h0 = hq * HQ
        out[b_idx, :, 12 * h0 : 12 * h0 + NT * HQ, :] = results[cid]["y"]
    return out


def kernel(**inputs) -> np.ndarray:
    nc = build_nc()
    in_maps = shard_inputs(inputs)
    res = run_bass_kernel_spmd(nc, in_maps, core_ids=list(range(NCORES)))
    return gather_outputs(res.results)



# revision 42
# speedup vs baseline: 1.5443x; 1.5443x over previous
"""TRN2 Bass kernel for nn_Construct_76484777607483.

Computes, for 12 input tensors x_i [B=2, C=256, H=64, W=256]:
    y_i = einsum('bchw,co->bohw', x_i, W)
interleaved over H (output row 12*h + i comes from tensor i, row h) into
out [2, 256, 768, 256], plus bias b[o] * count(row) where count is the
conv-transpose overlap multiplicity (ramp 1..12 at the top edge, 12 in the
middle, 12..1 at the bottom edge).

Sharding: 8 cores = (2 batches) x (4 h-quarters of 16 input rows). Each core
handles all 12 tensors for its 16 rows, so the row-interleave is assembled
on-chip and output DMA writes are fully contiguous per channel.

Design (fp8 DoubleRow): the 256-channel contraction runs on the PE in
fp8-e4m3 DoubleRow mode, which contracts BOTH 128-channel k-tiles in a single
instruction at 0.5 cycles/row — 4x the bf16/fp32r rate. Precision is restored
with a hi/lo residual split (x = x_hi + x_lo, W*64 = W_hi + W_lo, each half
fp8-quantized; W is pre-scaled by 64 to keep it out of the fp8-subnormal
range). Three DoubleRow passes accumulate W_hi.x_hi + W_hi.x_lo + W_lo.x_hi
into PSUM; the dropped lo.lo term and second-level residuals leave ~1.2e-3
relative error (measured on the real inputs) — better than a bf16 matmul.
PE time: 61.4us at 2.4GHz.

Each tensor pair's 4 PSUM banks (2 out-halves x 2 tensors) drain in ONE
scalar_tensor_tensor op, out = psum*(1/64) + bias, where the bias rides a
0-stride broadcast AP over a precomputed [128, mh, g*pair, hl, ip] table
(the conv-transpose count ramp is baked in, so edge groups are uniform).
Copies run on DVE (which cannot DMA) and Pool; one bf16 store per pair.
DMA queues: inputs are the packed hi/lo fp8 pair (2 B/pixel, same bytes as
bf16), outputs bf16, spread across SP/ACT/Pool near the ~7.9us/group cadence.
"""

import numpy as np
import ml_dtypes

import concourse.bacc as bacc
import concourse.tile as tile
import concourse.mybir as mybir
from concourse.bass_utils import run_bass_kernel_spmd

B, C, H, WD = 2, 256, 64, 256
NT = 12                 # stacked tensors
NCORES = 8
HQ = H // 4             # 16 input rows per core
NG = HQ // 2            # 8 groups of 2 rows
NP = NT // 2            # 6 tensor pairs
HOUT = NT * H           # 768
WS = 64.0               # weight pre-scale (keeps fp8(W) out of subnormals)

_F32 = mybir.dt.float32
_BF16 = mybir.dt.bfloat16
_F8 = mybir.dt.float8e4
_NPBF16 = ml_dtypes.bfloat16
_NPF8 = ml_dtypes.float8_e4m3

_NC_CACHE = {}


def build_nc():
    if "nc" in _NC_CACHE:
        return _NC_CACHE["nc"]
    nc = bacc.Bacc("TRN2", target_bir_lowering=False)
    # x layout [i, p(=c%128), ver(hi/lo), kh(=c//128), h, w]: one DMA per
    # (i, g) pulls both fp8 halves and both k-tiles as [128, ver, kh, hl, w].
    x_d = nc.declare_dram_parameter("x", [NT, 128, 2, 2, HQ, WD], _F8, isOutput=False)
    # w layout [ver, p(=k%128), ktile, mh, m]: SBUF tile [128, ktile, m] is
    # the DoubleRow lhsT (contracts both k-tiles in one matmul).
    w_d = nc.declare_dram_parameter("w", [2, 128, 2, 2, 128], _F8, isOutput=False)
    # bias tables per (scale-ver, mh, col), col = i*HQ + h:
    # ver 0 = 64*b*count (DVE tensor_scalar), ver 1 = b*count (ACT activation)
    bv_d = nc.declare_dram_parameter("bv", [128, 2, 2, NT * HQ], _F32, isOutput=False)
    # y layout [mh, p, h, i, w]: channel o = mh*128+p; (h, i) row-major is the
    # interleaved output row 12h+i.
    y_d = nc.declare_dram_parameter("y", [2, 128, HQ, NT, WD], _BF16, isOutput=True)

    dr = mybir.MatmulPerfMode.DoubleRow

    with tile.TileContext(nc) as tc:
        with (
            tc.tile_pool(name="const", bufs=1) as cpool,
            tc.tile_pool(name="xin", bufs=18) as inpool,
            tc.tile_pool(name="obuf", bufs=8) as outpool,
            tc.tile_pool(name="ps", bufs=4, space="PSUM") as pspool,
        ):
            wt = [
                [cpool.tile([128, 2, 128], _F8, name=f"w{v}{mh}") for mh in range(2)]
                for v in range(2)
            ]
            # spread the tiny weight loads across all three DMA queues so the
            # first matmul isn't serialized behind one queue's const chain
            _weng = [nc.sync, nc.scalar, nc.gpsimd, nc.sync]
            for v in range(2):
                for mh in range(2):
                    _weng[2 * v + mh].dma_start(
                        out=wt[v][mh][:], in_=w_d[v, :, :, mh, :]
                    )
            bvt = cpool.tile([128, 2, 2, NT * HQ], _F32, name="bv")
            # Pool has no g=0 input DMAs, so the bias table rides its queue
            # early and lands (~3.6us) just before the first copy needs it
            nc.gpsimd.dma_start(out=bvt[:], in_=bv_d[:])

            # GPSIMD cannot access PSUM on hardware, so the PSUM->SBUF copies
            # run ONLY on DVE (which cannot DMA) and ACT; SP/Pool carry almost
            # all DMA traffic. Copies: 12 half-tile ops per group, the two
            # halves of a pair on DIFFERENT engines so each 2-bank PSUM tile
            # drains fast against the PE's ~1.28us/pair fill cadence.
            _dve_slots = {6: [0, 2, 4, 6, 8, 10], 7: [0, 2, 4, 6, 8, 9, 11]}
            _dve_per_g = [6, 7, 7, 7, 7, 7, 7, 6]
            copy_rot = []
            for _g in range(NG):
                rot = ["act"] * 12
                for k in _dve_slots[_dve_per_g[_g]]:
                    rot[k] = "dve"
                copy_rot.append(rot)
            in_rot_mid = ["sp", "pool", "sp", "pool", "sp", "pool",
                          "sp", "pool", "sp", "pool", "sp", "pool"]
            out_rot_even = ["sp", "pool", "pool", "sp", "act", "pool",
                            "sp", "pool", "sp", "act", "act", "sp"]
            out_rot_odd = ["sp", "pool", "pool", "sp", "act", "pool",
                           "sp", "pool", "sp", "act", "pool", "act"]
            out_rot_bnd = ["sp", "pool", "pool", "sp", "sp", "pool",
                           "sp", "pool", "sp", "pool", "pool", "sp"]
            in_rot, out_rot = [], []
            for _g in range(NG):
                irot = list(in_rot_mid)
                if _g % 2 == 0:
                    # shift one input fetch per even group from Pool to ACT
                    irot[11] = "act"
                in_rot.append(irot)
                if _g in (0, NG - 1):
                    out_rot.append(list(out_rot_bnd))
                elif _g % 2 == 0:
                    out_rot.append(list(out_rot_even))
                else:
                    rot = list(out_rot_odd)
                    if _g in (3, 5):
                        rot[0] = "act"
                    out_rot.append(rot)
            _ENG = {"pool": nc.gpsimd, "act": nc.scalar, "sp": nc.sync,
                    "dve": nc.vector}

            for g in range(NG):
                xins = {}
                for i in range(NT):
                    xin = inpool.tile(
                        [128, 2, 2, 2, WD], _F8, name=f"xin{g}_{i}", tag="xin"
                    )
                    _ENG[in_rot[g][i]].dma_start(
                        out=xin[:], in_=x_d[i, :, :, :, 2 * g : 2 * g + 2, :]
                    )
                    xins[i] = xin

                for pr in range(NP):
                    i0 = 2 * pr
                    last_pair = g == NG - 1 and pr == NP - 1
                    boundary = g in (0, NG - 1)
                    for mh in range(2):
                        # one 2-bank PSUM tile per (pair, out-half): [128,ip,hl,w]
                        ps = pspool.tile(
                            [128, 2, 2, WD], _F32, name=f"ps{g}_{pr}_{mh}", tag="ps"
                        )
                        for ip in range(2):
                            rhs_hi = xins[i0 + ip][:, 0]
                            rhs_lo = xins[i0 + ip][:, 1]
                            nc.tensor.matmul(
                                ps[:, ip], wt[0][mh][:], rhs_hi,
                                start=True, stop=False, perf_mode=dr,
                            )
                            nc.tensor.matmul(
                                ps[:, ip], wt[0][mh][:], rhs_lo,
                                start=False, stop=False, perf_mode=dr,
                            )
                            nc.tensor.matmul(
                                ps[:, ip], wt[1][mh][:], rhs_hi,
                                start=False, stop=True, perf_mode=dr,
                            )
                        ob = outpool.tile(
                            [128, 2, 2, WD], _BF16, name=f"ob{g}_{pr}_{mh}", tag="ob"
                        )

                        def bcopy(cname, dst, srcap, col):
                            if cname == "dve":
                                # out = (psum + 64*b*count) * (1/64)
                                nc.vector.tensor_scalar(
                                    dst, srcap,
                                    bvt[:, 0, mh, col : col + 1], 1.0 / WS,
                                    mybir.AluOpType.add, mybir.AluOpType.mult,
                                )
                            else:
                                # out = Identity(psum * (1/64) + b*count)
                                nc.scalar.activation(
                                    dst, srcap,
                                    mybir.ActivationFunctionType.Identity,
                                    bias=bvt[:, 1, mh, col : col + 1],
                                    scale=1.0 / WS,
                                )

                        src = ps[:].transpose([0, 2, 1, 3])  # -> [128,hl,ip,w]
                        if last_pair:
                            # drain tail: split the final copies/stores at row
                            # granularity; count varies per tensor on the edge
                            # row, so that row splits per tensor as well
                            hv, hu = 1, 0
                            for ip in range(2):
                                bcopy("dve" if ip == 0 else "act",
                                      ob[:, hv, ip], ps[:, ip, hv],
                                      (i0 + ip) * HQ + 2 * g + hv)
                            bcopy("dve", ob[:, hu], ps[:, :, hu],
                                  i0 * HQ + 2 * g + hu)
                            for hl in range(2):
                                (nc.sync if hl else nc.gpsimd).dma_start(
                                    out=y_d[mh, :, 2 * g + hl, i0 : i0 + 2, :],
                                    in_=ob[:, hl],
                                )
                        else:
                            cname = copy_rot[g][2 * pr + mh]
                            oeng = _ENG[out_rot[g][2 * pr + mh]]
                            if boundary:
                                # one row of the pair sits on the count ramp
                                # (bias varies per tensor): copy it per tensor
                                hv = 0 if g == 0 else 1
                                hu = 1 - hv
                                for ip in range(2):
                                    bcopy(cname, ob[:, hv, ip], ps[:, ip, hv],
                                          (i0 + ip) * HQ + 2 * g + hv)
                                bcopy(cname, ob[:, hu], ps[:, :, hu],
                                      i0 * HQ + 2 * g + hu)
                            else:
                                bcopy(cname, ob[:], src, i0 * HQ + 2 * g)
                            oeng.dma_start(
                                out=y_d[mh, :, 2 * g : 2 * g + 2, i0 : i0 + 2, :],
                                in_=ob[:],
                            )
    nc.finalize()
    _NC_CACHE["nc"] = nc
    return nc


def _counts() -> np.ndarray:
    """count[r] for output row r (conv-transpose bias multiplicity)."""
    r = np.arange(HOUT)
    return (np.minimum(11, r) - np.maximum(0, r - (HOUT - NT)) + 1).astype(np.float32)


def shard_inputs(inputs: dict) -> list[dict]:
    # fp8 hi/lo split of the inputs (full tensors once, then slice per core)
    x_hilo = []
    for i in range(NT):
        xf = np.asarray(inputs[f"x{i}"], dtype=np.float32)
        hi = xf.astype(_NPF8)
        lo = (xf - hi.astype(np.float32)).astype(_NPF8)
        x_hilo.append((hi, lo))
    wf = np.asarray(inputs["W"], dtype=np.float32) * WS  # [c, o]
    w_hi = wf.astype(_NPF8)
    w_lo = (wf - w_hi.astype(np.float32)).astype(_NPF8)
    # w_d[ver, p, ktile, mh, m]: c = ktile*128 + p, o = mh*128 + m
    w_pack = np.empty((2, 128, 2, 2, 128), dtype=_NPF8)
    for v, wv in enumerate((w_hi, w_lo)):
        w_pack[v] = wv.reshape(2, 128, 2, 128).transpose(1, 0, 2, 3)
    b = np.asarray(inputs["b"], dtype=np.float32)
    counts = _counts()
    in_maps = []
    for cid in range(NCORES):
        b_idx, hq = divmod(cid, 4)
        h0 = hq * HQ
        x_core = np.empty((NT, 128, 2, 2, HQ, WD), dtype=_NPF8)
        for i in range(NT):
            for v in range(2):
                # [C, HQ, WD] -> [kh, 128, HQ, WD] -> [128, kh, HQ, WD]
                blk = x_hilo[i][v][b_idx, :, h0 : h0 + HQ, :].reshape(2, 128, HQ, WD)
                x_core[i, :, v] = blk.transpose(1, 0, 2, 3)
        # bv[p, ver, mh, i*HQ + hl] = scale * b[mh*128+p] * count(12*(h0+hl) + i)
        # with scale = WS for the DVE table (ver 0), 1 for ACT (ver 1)
        i_idx = np.arange(NT)[:, None]
        hl_idx = np.arange(HQ)[None, :]
        cnt = counts[12 * (h0 + hl_idx) + i_idx].reshape(NT * HQ)  # [192]
        base = b.reshape(2, 128).T[:, None, :, None] * cnt[None, None, None, :]
        bv = (np.array([WS, 1.0], dtype=np.float32)[None, :, None, None]
              * base).astype(np.float32)  # [128, 2, 2, 192]
        in_maps.append({"x": x_core, "w": w_pack, "bv": bv})
    return in_maps


def gather_outputs(results: list[dict]) -> np.ndarray:
    out = np.empty((B, C, HOUT, WD), dtype=np.float32)
    for cid in range(NCORES):
        b_idx, hq = divmod(cid, 4)
        h0 = hq * HQ
        y = np.asarray(results[cid]["y"])  # [2, 128, HQ, NT, WD] bf16
        out[b_idx, :, 12 * h0 : 12 * h0 + NT * HQ, :] = (
            y.reshape(C, NT * HQ, WD).astype(np.float32)
        )
    return out


def kernel(**inputs) -> np.ndarray:
    nc = build_nc()
    in_maps = shard_inputs(inputs)
    res = run_bass_kernel_spmd(nc, in_maps, core_ids=list(range(NCORES)))
    return gather_outputs(res.results)


# revision 47
# speedup vs baseline: 1.5892x; 1.0291x over previous
"""TRN2 Bass kernel for nn_Construct_76484777607483.

Computes, for 12 input tensors x_i [B=2, C=256, H=64, W=256]:
    y_i = einsum('bchw,co->bohw', x_i, W)
interleaved over H (output row 12*h + i comes from tensor i, row h) into
out [2, 256, 768, 256], plus bias b[o] * count(row) where count is the
conv-transpose overlap multiplicity (ramp 1..12 at the top edge, 12 in the
middle, 12..1 at the bottom edge).

Sharding: 8 cores = (2 batches) x (4 h-quarters of 16 input rows). Each core
handles all 12 tensors for its 16 rows, so the row-interleave is assembled
on-chip and output DMA writes are fully contiguous per channel.

Design (fp8 DoubleRow): the 256-channel contraction runs on the PE in
fp8-e4m3 DoubleRow mode, which contracts BOTH 128-channel k-tiles in a single
instruction at 0.5 cycles/row — 4x the bf16/fp32r rate. Precision is restored
with a hi/lo residual split (x = x_hi + x_lo, W*64 = W_hi + W_lo, each half
fp8-quantized; W is pre-scaled by 64 to keep it out of the fp8-subnormal
range). Three DoubleRow passes accumulate W_hi.x_hi + W_hi.x_lo + W_lo.x_hi
into PSUM; the dropped lo.lo term and second-level residuals leave ~1.2e-3
relative error (measured on the real inputs) — better than a bf16 matmul.
PE time: 61.4us at 2.4GHz.

PSUM->SBUF drains apply (psum + 64*b*count)*(1/64) and convert to bf16.
GPSIMD cannot touch PSUM on TRN2, so these copies run only on DVE (via
tensor_scalar; DVE cannot DMA) and ACT (via activation Identity with
scale=1/64 and an unscaled bias table). SP and Pool carry nearly all DMA:
inputs are the packed hi/lo fp8 pair (2 B/pixel, same bytes as bf16),
outputs bf16 (upcast on host). Engine rotations keep every queue near the
~8.3us/group cadence; measured on hardware: rel_err 1.9e-3.
"""

import numpy as np
import ml_dtypes

import concourse.bacc as bacc
import concourse.tile as tile
import concourse.mybir as mybir
from concourse.bass_utils import run_bass_kernel_spmd

B, C, H, WD = 2, 256, 64, 256
NT = 12                 # stacked tensors
NCORES = 8
HQ = H // 4             # 16 input rows per core
NG = HQ // 2            # 8 groups of 2 rows
NP = NT // 2            # 6 tensor pairs
HOUT = NT * H           # 768
WS = 64.0               # weight pre-scale (keeps fp8(W) out of subnormals)

_F32 = mybir.dt.float32
_BF16 = mybir.dt.bfloat16
_F8 = mybir.dt.float8e4
_NPBF16 = ml_dtypes.bfloat16
_NPF8 = ml_dtypes.float8_e4m3

_NC_CACHE = {}


def build_nc():
    if "nc" in _NC_CACHE:
        return _NC_CACHE["nc"]
    nc = bacc.Bacc("TRN2", target_bir_lowering=False)
    # x layout [i, p(=c%128), ver(hi/lo), kh(=c//128), h, w]: one DMA per
    # (i, g) pulls both fp8 halves and both k-tiles as [128, ver, kh, hl, w].
    x_d = nc.declare_dram_parameter("x", [NT, 128, 2, 2, HQ, WD], _F8, isOutput=False)
    # w layout [ver, p(=k%128), ktile, mh, m]: SBUF tile [128, ktile, m] is
    # the DoubleRow lhsT (contracts both k-tiles in one matmul).
    w_d = nc.declare_dram_parameter("w", [2, 128, 2, 2, 128], _F8, isOutput=False)
    # bias tables per (scale-ver, mh, col), col = i*HQ + h:
    # ver 0 = 64*b*count (DVE tensor_scalar), ver 1 = b*count (ACT activation)
    bv_d = nc.declare_dram_parameter("bv", [128, 2, 2, NT * HQ], _F32, isOutput=False)
    # y layout [mh, p, h, i, w]: channel o = mh*128+p; (h, i) row-major is the
    # interleaved output row 12h+i.
    y_d = nc.declare_dram_parameter("y", [2, 128, HQ, NT, WD], _BF16, isOutput=True)

    dr = mybir.MatmulPerfMode.DoubleRow

    with tile.TileContext(nc) as tc:
        with (
            tc.tile_pool(name="const", bufs=1) as cpool,
            tc.tile_pool(name="xin", bufs=24) as inpool,
            tc.tile_pool(name="obuf", bufs=12) as outpool,
            tc.tile_pool(name="ps", bufs=4, space="PSUM") as pspool,
        ):
            wt = [
                [cpool.tile([128, 2, 128], _F8, name=f"w{v}{mh}") for mh in range(2)]
                for v in range(2)
            ]
            # spread the tiny weight loads across all three DMA queues so the
            # first matmul isn't serialized behind one queue's const chain
            _weng = [nc.sync, nc.scalar, nc.gpsimd, nc.sync]
            for v in range(2):
                for mh in range(2):
                    _weng[2 * v + mh].dma_start(
                        out=wt[v][mh][:], in_=w_d[v, :, :, mh, :]
                    )
            bvt = cpool.tile([128, 2, 2, NT * HQ], _F32, name="bv")
            # Pool has no g=0 input DMAs, so the bias table rides its queue
            # early and lands (~3.6us) just before the first copy needs it
            nc.gpsimd.dma_start(out=bvt[:], in_=bv_d[:])

            # GPSIMD cannot access PSUM on hardware, so the PSUM->SBUF copies
            # run ONLY on DVE (which cannot DMA) and ACT; SP/Pool carry almost
            # all DMA traffic. Copies: 12 half-tile ops per group, the two
            # halves of a pair on DIFFERENT engines so each 2-bank PSUM tile
            # drains fast against the PE's ~1.28us/pair fill cadence.
            _dve_slots = {6: [0, 2, 4, 6, 8, 10], 7: [0, 2, 4, 6, 8, 9, 11]}
            _dve_per_g = [6, 7, 7, 7, 7, 7, 7, 6]
            copy_rot = []
            for _g in range(NG):
                rot = ["act"] * 12
                for k in _dve_slots[_dve_per_g[_g]]:
                    rot[k] = "dve"
                copy_rot.append(rot)
            in_rot_mid = ["sp", "pool", "sp", "pool", "sp", "pool",
                          "sp", "pool", "sp", "pool", "sp", "pool"]
            out_rot_even = ["sp", "pool", "pool", "sp", "act", "pool",
                            "sp", "pool", "sp", "act", "act", "sp"]
            out_rot_odd = ["sp", "pool", "pool", "sp", "act", "pool",
                           "sp", "pool", "sp", "act", "pool", "act"]
            out_rot_bnd = ["sp", "pool", "pool", "sp", "sp", "pool",
                           "sp", "pool", "sp", "pool", "pool", "sp"]
            in_rot, out_rot = [], []
            for _g in range(NG):
                irot = list(in_rot_mid)
                if _g % 2 == 0:
                    # shift one input fetch per even group from Pool to ACT
                    irot[11] = "act"
                in_rot.append(irot)
                if _g in (0, NG - 1):
                    out_rot.append(list(out_rot_bnd))
                elif _g % 2 == 0:
                    out_rot.append(list(out_rot_even))
                else:
                    rot = list(out_rot_odd)
                    if _g in (3, 5):
                        rot[0] = "act"
                    out_rot.append(rot)
            _ENG = {"pool": nc.gpsimd, "act": nc.scalar, "sp": nc.sync,
                    "dve": nc.vector}

            for g in range(NG):
                xins = {}
                for i in range(NT):
                    xin = inpool.tile(
                        [128, 2, 2, 2, WD], _F8, name=f"xin{g}_{i}", tag="xin"
                    )
                    _ENG[in_rot[g][i]].dma_start(
                        out=xin[:], in_=x_d[i, :, :, :, 2 * g : 2 * g + 2, :]
                    )
                    xins[i] = xin

                for pr in range(NP):
                    i0 = 2 * pr
                    last_pair = g == NG - 1 and pr == NP - 1
                    boundary = g in (0, NG - 1)
                    for mh in range(2):
                        # one 2-bank PSUM tile per (pair, out-half): [128,ip,hl,w]
                        ps = pspool.tile(
                            [128, 2, 2, WD], _F32, name=f"ps{g}_{pr}_{mh}", tag="ps"
                        )
                        for ip in range(2):
                            rhs_hi = xins[i0 + ip][:, 0]
                            rhs_lo = xins[i0 + ip][:, 1]
                            nc.tensor.matmul(
                                ps[:, ip], wt[0][mh][:], rhs_hi,
                                start=True, stop=False, perf_mode=dr,
                            )
                            nc.tensor.matmul(
                                ps[:, ip], wt[0][mh][:], rhs_lo,
                                start=False, stop=False, perf_mode=dr,
                            )
                            nc.tensor.matmul(
                                ps[:, ip], wt[1][mh][:], rhs_hi,
                                start=False, stop=True, perf_mode=dr,
                            )
                        ob = outpool.tile(
                            [128, 2, 2, WD], _BF16, name=f"ob{g}_{pr}_{mh}", tag="ob"
                        )

                        def bcopy(cname, dst, srcap, col):
                            if cname == "dve":
                                # out = (psum + 64*b*count) * (1/64)
                                nc.vector.tensor_scalar(
                                    dst, srcap,
                                    bvt[:, 0, mh, col : col + 1], 1.0 / WS,
                                    mybir.AluOpType.add, mybir.AluOpType.mult,
                                )
                            else:
                                # out = Identity(psum * (1/64) + b*count)
                                nc.scalar.activation(
                                    dst, srcap,
                                    mybir.ActivationFunctionType.Identity,
                                    bias=bvt[:, 1, mh, col : col + 1],
                                    scale=1.0 / WS,
                                )

                        src = ps[:].transpose([0, 2, 1, 3])  # -> [128,hl,ip,w]
                        if last_pair:
                            # drain tail: split the final copies/stores at row
                            # granularity; count varies per tensor on the edge
                            # row, so that row splits per tensor as well
                            hv, hu = 1, 0
                            for ip in range(2):
                                bcopy("dve" if ip == 0 else "act",
                                      ob[:, hv, ip], ps[:, ip, hv],
                                      (i0 + ip) * HQ + 2 * g + hv)
                            bcopy("dve", ob[:, hu], ps[:, :, hu],
                                  i0 * HQ + 2 * g + hu)
                            for hl in range(2):
                                (nc.sync if hl else nc.gpsimd).dma_start(
                                    out=y_d[mh, :, 2 * g + hl, i0 : i0 + 2, :],
                                    in_=ob[:, hl],
                                )
                        else:
                            cname = copy_rot[g][2 * pr + mh]
                            oeng = _ENG[out_rot[g][2 * pr + mh]]
                            if boundary:
                                # one row of the pair sits on the count ramp
                                # (bias varies per tensor): copy it per tensor
                                hv = 0 if g == 0 else 1
                                hu = 1 - hv
                                for ip in range(2):
                                    bcopy(cname, ob[:, hv, ip], ps[:, ip, hv],
                                          (i0 + ip) * HQ + 2 * g + hv)
                                bcopy(cname, ob[:, hu], ps[:, :, hu],
                                      i0 * HQ + 2 * g + hu)
                            else:
                                bcopy(cname, ob[:], src, i0 * HQ + 2 * g)
                            oeng.dma_start(
                                out=y_d[mh, :, 2 * g : 2 * g + 2, i0 : i0 + 2, :],
                                in_=ob[:],
                            )
    nc.finalize()
    _NC_CACHE["nc"] = nc
    return nc


def _counts() -> np.ndarray:
    """count[r] for output row r (conv-transpose bias multiplicity)."""
    r = np.arange(HOUT)
    return (np.minimum(11, r) - np.maximum(0, r - (HOUT - NT)) + 1).astype(np.float32)


def shard_inputs(inputs: dict) -> list[dict]:
    # fp8 hi/lo split of the inputs (full tensors once, then slice per core)
    x_hilo = []
    for i in range(NT):
        xf = np.asarray(inputs[f"x{i}"], dtype=np.float32)
        hi = xf.astype(_NPF8)
        lo = (xf - hi.astype(np.float32)).astype(_NPF8)
        x_hilo.append((hi, lo))
    wf = np.asarray(inputs["W"], dtype=np.float32) * WS  # [c, o]
    w_hi = wf.astype(_NPF8)
    w_lo = (wf - w_hi.astype(np.float32)).astype(_NPF8)
    # w_d[ver, p, ktile, mh, m]: c = ktile*128 + p, o = mh*128 + m
    w_pack = np.empty((2, 128, 2, 2, 128), dtype=_NPF8)
    for v, wv in enumerate((w_hi, w_lo)):
        w_pack[v] = wv.reshape(2, 128, 2, 128).transpose(1, 0, 2, 3)
    b = np.asarray(inputs["b"], dtype=np.float32)
    counts = _counts()
    in_maps = []
    for cid in range(NCORES):
        b_idx, hq = divmod(cid, 4)
        h0 = hq * HQ
        x_core = np.empty((NT, 128, 2, 2, HQ, WD), dtype=_NPF8)
        for i in range(NT):
            for v in range(2):
                # [C, HQ, WD] -> [kh, 128, HQ, WD] -> [128, kh, HQ, WD]
                blk = x_hilo[i][v][b_idx, :, h0 : h0 + HQ, :].reshape(2, 128, HQ, WD)
                x_core[i, :, v] = blk.transpose(1, 0, 2, 3)
        # bv[p, ver, mh, i*HQ + hl] = scale * b[mh*128+p] * count(12*(h0+hl) + i)
        # with scale = WS for the DVE table (ver 0), 1 for ACT (ver 1)
        i_idx = np.arange(NT)[:, None]
        hl_idx = np.arange(HQ)[None, :]
        cnt = counts[12 * (h0 + hl_idx) + i_idx].reshape(NT * HQ)  # [192]
        base = b.reshape(2, 128).T[:, None, :, None] * cnt[None, None, None, :]
        bv = (np.array([WS, 1.0], dtype=np.float32)[None, :, None, None]
              * base).astype(np.float32)  # [128, 2, 2, 192]
        in_maps.append({"x": x_core, "w": w_pack, "bv": bv})
    return in_maps


def gather_outputs(results: list[dict]) -> np.ndarray:
    out = np.empty((B, C, HOUT, WD), dtype=np.float32)
    for cid in range(NCORES):
        b_idx, hq = divmod(cid, 4)
        h0 = hq * HQ
        y = np.asarray(results[cid]["y"])  # [2, 128, HQ, NT, WD] bf16
        out[b_idx, :, 12 * h0 : 12 * h0 + NT * HQ, :] = (
            y.reshape(C, NT * HQ, WD).astype(np.float32)
        )
    return out


def kernel(**inputs) -> np.ndarray:
    nc = build_nc()
    in_maps = shard_inputs(inputs)
    res = run_bass_kernel_spmd(nc, in_maps, core_ids=list(range(NCORES)))
    return gather_outputs(res.results)


# revision 60
# speedup vs baseline: 1.5969x; 1.0048x over previous
"""TRN2 Bass kernel for nn_Construct_76484777607483.

Computes, for 12 input tensors x_i [B=2, C=256, H=64, W=256]:
    y_i = einsum('bchw,co->bohw', x_i, W)
interleaved over H (output row 12*h + i comes from tensor i, row h) into
out [2, 256, 768, 256], plus bias b[o] * count(row) where count is the
conv-transpose overlap multiplicity (ramp 1..12 at the top edge, 12 in the
middle, 12..1 at the bottom edge).

Sharding: 8 cores = (2 batches) x (4 h-quarters of 16 input rows). Each core
handles all 12 tensors for its 16 rows, so the row-interleave is assembled
on-chip and output DMA writes are fully contiguous per channel.

Design (fp8 DoubleRow): the 256-channel contraction runs on the PE in
fp8-e4m3 DoubleRow mode, which contracts BOTH 128-channel k-tiles in a single
instruction at 0.5 cycles/row — 4x the bf16/fp32r rate. Precision is restored
with a hi/lo residual split (x = x_hi + x_lo, W*64 = W_hi + W_lo, each half
fp8-quantized; W is pre-scaled by 64 to keep it out of the fp8-subnormal
range). Three DoubleRow passes accumulate W_hi.x_hi + W_hi.x_lo + W_lo.x_hi
into PSUM; the dropped lo.lo term and second-level residuals leave ~1.2e-3
relative error (measured on the real inputs) — better than a bf16 matmul.
PE time: 61.4us at 2.4GHz.

PSUM->SBUF drains apply (psum + 64*b*count)*(1/64) and convert to bf16.
GPSIMD cannot touch PSUM on TRN2, so these copies run only on DVE (via
tensor_scalar; DVE cannot DMA) and ACT (via activation Identity with
scale=1/64 and an unscaled bias table). SP and Pool carry nearly all DMA:
inputs are the packed hi/lo fp8 pair (2 B/pixel, same bytes as bf16),
outputs bf16 (upcast on host). Engine rotations keep every queue near the
~8.3us/group cadence; measured on hardware: rel_err 1.9e-3.
"""

import numpy as np
import ml_dtypes

import concourse.bacc as bacc
import concourse.tile as tile
import concourse.mybir as mybir
from concourse.bass_utils import run_bass_kernel_spmd

B, C, H, WD = 2, 256, 64, 256
NT = 12                 # stacked tensors
NCORES = 8
HQ = H // 4             # 16 input rows per core
NG = HQ // 2            # 8 groups of 2 rows
NP = NT // 2            # 6 tensor pairs
HOUT = NT * H           # 768
WS = 64.0               # weight pre-scale (keeps fp8(W) out of subnormals)

_F32 = mybir.dt.float32
_BF16 = mybir.dt.bfloat16
_F8 = mybir.dt.float8e4
_NPBF16 = ml_dtypes.bfloat16
_NPF8 = ml_dtypes.float8_e4m3

_NC_CACHE = {}


def build_nc():
    if "nc" in _NC_CACHE:
        return _NC_CACHE["nc"]
    nc = bacc.Bacc("TRN2", target_bir_lowering=False)
    # x layout [i, p(=c%128), ver(hi/lo), kh(=c//128), h, w]: one DMA per
    # (i, g) pulls both fp8 halves and both k-tiles as [128, ver, kh, hl, w].
    x_d = nc.declare_dram_parameter("x", [NT, 128, 2, 2, HQ, WD], _F8, isOutput=False)
    # w layout [ver, p(=k%128), ktile, mh, m]: SBUF tile [128, ktile, m] is
    # the DoubleRow lhsT (contracts both k-tiles in one matmul).
    w_d = nc.declare_dram_parameter("w", [2, 128, 2, 2, 128], _F8, isOutput=False)
    # bias tables per (scale-ver, mh, col), col = i*HQ + h:
    # ver 0 = 64*b*count (DVE tensor_scalar), ver 1 = b*count (ACT activation)
    bv_d = nc.declare_dram_parameter("bv", [128, 2, 2, NT * HQ], _F32, isOutput=False)
    # y layout [mh, p, h, i, w]: channel o = mh*128+p; (h, i) row-major is the
    # interleaved output row 12h+i.
    y_d = nc.declare_dram_parameter("y", [2, 128, HQ, NT, WD], _BF16, isOutput=True)

    dr = mybir.MatmulPerfMode.DoubleRow

    SKIP_LO_G = ()

    with tile.TileContext(nc) as tc:
        with (
            tc.tile_pool(name="const", bufs=1) as cpool,
            tc.tile_pool(name="xin", bufs=24) as inpool,
            tc.tile_pool(name="xinh", bufs=12) as inpool_h,
            tc.tile_pool(name="obuf", bufs=12) as outpool,
            tc.tile_pool(name="ps", bufs=4, space="PSUM") as pspool,
        ):
            wt = [
                [cpool.tile([128, 2, 128], _F8, name=f"w{v}{mh}") for mh in range(2)]
                for v in range(2)
            ]
            # spread the tiny weight loads across all three DMA queues so the
            # first matmul isn't serialized behind one queue's const chain
            _weng = [nc.sync, nc.scalar, nc.gpsimd, nc.sync]
            for v in range(2):
                for mh in range(2):
                    _weng[2 * v + mh].dma_start(
                        out=wt[v][mh][:], in_=w_d[v, :, :, mh, :]
                    )
            bvt = cpool.tile([128, 2, 2, NT * HQ], _F32, name="bv")
            # Pool has no g=0 input DMAs, so the bias table rides its queue
            # early and lands (~3.6us) just before the first copy needs it
            nc.gpsimd.dma_start(out=bvt[:], in_=bv_d[:])

            # GPSIMD cannot access PSUM on hardware, so the PSUM->SBUF copies
            # run ONLY on DVE (which cannot DMA) and ACT; SP/Pool carry almost
            # all DMA traffic. Copies: 12 half-tile ops per group, the two
            # halves of a pair on DIFFERENT engines so each 2-bank PSUM tile
            # drains fast against the PE's ~1.28us/pair fill cadence.
            _dve_slots = {6: [0, 2, 4, 6, 8, 10], 7: [0, 2, 4, 6, 8, 9, 11]}
            _dve_per_g = [6, 7, 7, 7, 7, 7, 7, 6]
            copy_rot = []
            for _g in range(NG):
                rot = ["act"] * 12
                for k in _dve_slots[_dve_per_g[_g]]:
                    rot[k] = "dve"
                copy_rot.append(rot)
            in_rot_mid = ["sp", "pool", "sp", "pool", "sp", "pool",
                          "sp", "pool", "sp", "pool", "sp", "pool"]
            out_rot_even = ["sp", "pool", "pool", "sp", "act", "pool",
                            "sp", "pool", "sp", "act", "act", "sp"]
            out_rot_odd = ["sp", "pool", "pool", "sp", "act", "pool",
                           "sp", "pool", "sp", "act", "pool", "act"]
            out_rot_bnd = ["sp", "pool", "pool", "sp", "sp", "pool",
                           "sp", "pool", "sp", "pool", "pool", "sp"]
            in_rot, out_rot = [], []
            for _g in range(NG):
                irot = list(in_rot_mid)
                if _g % 2 == 0:
                    # shift one input fetch per even group from Pool to ACT
                    irot[11] = "act"
                in_rot.append(irot)
                if _g in (0, NG - 1):
                    out_rot.append(list(out_rot_bnd))
                elif _g % 2 == 0:
                    out_rot.append(list(out_rot_even))
                else:
                    rot = list(out_rot_odd)
                    if _g in (3, 5):
                        rot[0] = "act"
                    out_rot.append(rot)
            _ENG = {"pool": nc.gpsimd, "act": nc.scalar, "sp": nc.sync,
                    "dve": nc.vector}

            for g in range(NG):
                skip_lo = g in SKIP_LO_G
                xins = {}
                for i in range(NT):
                    if skip_lo:
                        xin = inpool_h.tile(
                            [128, 1, 2, 2, WD], _F8, name=f"xin{g}_{i}", tag="xinh"
                        )
                        _ENG[in_rot[g][i]].dma_start(
                            out=xin[:], in_=x_d[i, :, 0:1, :, 2 * g : 2 * g + 2, :]
                        )
                    else:
                        xin = inpool.tile(
                            [128, 2, 2, 2, WD], _F8, name=f"xin{g}_{i}", tag="xin"
                        )
                        _ENG[in_rot[g][i]].dma_start(
                            out=xin[:], in_=x_d[i, :, :, :, 2 * g : 2 * g + 2, :]
                        )
                    xins[i] = xin

                for pr in range(NP):
                    i0 = 2 * pr
                    last_pair = g == NG - 1 and pr == NP - 1
                    boundary = g in (0, NG - 1)
                    for mh in range(2):
                        # one 2-bank PSUM tile per (pair, out-half): [128,ip,hl,w]
                        ps = pspool.tile(
                            [128, 2, 2, WD], _F32, name=f"ps{g}_{pr}_{mh}", tag="ps"
                        )
                        for ip in range(2):
                            rhs_hi = xins[i0 + ip][:, 0]
                            nc.tensor.matmul(
                                ps[:, ip], wt[0][mh][:], rhs_hi,
                                start=True, stop=False, perf_mode=dr,
                            )
                            if not skip_lo:
                                nc.tensor.matmul(
                                    ps[:, ip], wt[0][mh][:], xins[i0 + ip][:, 1],
                                    start=False, stop=False, perf_mode=dr,
                                )
                            nc.tensor.matmul(
                                ps[:, ip], wt[1][mh][:], rhs_hi,
                                start=False, stop=True, perf_mode=dr,
                            )
                        ob = outpool.tile(
                            [128, 2, 2, WD], _BF16, name=f"ob{g}_{pr}_{mh}", tag="ob"
                        )

                        def bcopy(cname, dst, srcap, col):
                            if cname == "dve":
                                # out = (psum + 64*b*count) * (1/64)
                                nc.vector.tensor_scalar(
                                    dst, srcap,
                                    bvt[:, 0, mh, col : col + 1], 1.0 / WS,
                                    mybir.AluOpType.add, mybir.AluOpType.mult,
                                )
                            else:
                                # out = Identity(psum * (1/64) + b*count)
                                nc.scalar.activation(
                                    dst, srcap,
                                    mybir.ActivationFunctionType.Identity,
                                    bias=bvt[:, 1, mh, col : col + 1],
                                    scale=1.0 / WS,
                                )

                        src = ps[:].transpose([0, 2, 1, 3])  # -> [128,hl,ip,w]
                        if last_pair:
                            # drain tail: split the final copies/stores at row
                            # granularity; count varies per tensor on the edge
                            # row, so that row splits per tensor as well
                            hv, hu = 1, 0
                            for ip in range(2):
                                bcopy("dve" if ip == 0 else "act",
                                      ob[:, hv, ip], ps[:, ip, hv],
                                      (i0 + ip) * HQ + 2 * g + hv)
                            bcopy("dve", ob[:, hu], ps[:, :, hu],
                                  i0 * HQ + 2 * g + hu)
                            for hl in range(2):
                                # HWDGE queues have 167ns less DMA-init
                                # latency than Pool's SWDGE on the tail chain
                                (nc.sync if hl else nc.scalar).dma_start(
                                    out=y_d[mh, :, 2 * g + hl, i0 : i0 + 2, :],
                                    in_=ob[:, hl],
                                )
                        else:
                            cname = copy_rot[g][2 * pr + mh]
                            oeng = _ENG[out_rot[g][2 * pr + mh]]
                            if boundary:
                                # one row of the pair sits on the count ramp
                                # (bias varies per tensor): copy it per tensor
                                hv = 0 if g == 0 else 1
                                hu = 1 - hv
                                for ip in range(2):
                                    bcopy(cname, ob[:, hv, ip], ps[:, ip, hv],
                                          (i0 + ip) * HQ + 2 * g + hv)
                                bcopy(cname, ob[:, hu], ps[:, :, hu],
                                      i0 * HQ + 2 * g + hu)
                            else:
                                bcopy(cname, ob[:], src, i0 * HQ + 2 * g)
                            oeng.dma_start(
                                out=y_d[mh, :, 2 * g : 2 * g + 2, i0 : i0 + 2, :],
                                in_=ob[:],
                            )
    nc.finalize()
    _NC_CACHE["nc"] = nc
    return nc


def _counts() -> np.ndarray:
    """count[r] for output row r (conv-transpose bias multiplicity)."""
    r = np.arange(HOUT)
    return (np.minimum(11, r) - np.maximum(0, r - (HOUT - NT)) + 1).astype(np.float32)


def shard_inputs(inputs: dict) -> list[dict]:
    # fp8 hi/lo split of the inputs (full tensors once, then slice per core)
    x_hilo = []
    for i in range(NT):
        xf = np.asarray(inputs[f"x{i}"], dtype=np.float32)
        hi = xf.astype(_NPF8)
        lo = (xf - hi.astype(np.float32)).astype(_NPF8)
        x_hilo.append((hi, lo))
    wf = np.asarray(inputs["W"], dtype=np.float32) * WS  # [c, o]
    w_hi = wf.astype(_NPF8)
    w_lo = (wf - w_hi.astype(np.float32)).astype(_NPF8)
    # w_d[ver, p, ktile, mh, m]: c = ktile*128 + p, o = mh*128 + m
    w_pack = np.empty((2, 128, 2, 2, 128), dtype=_NPF8)
    for v, wv in enumerate((w_hi, w_lo)):
        w_pack[v] = wv.reshape(2, 128, 2, 128).transpose(1, 0, 2, 3)
    b = np.asarray(inputs["b"], dtype=np.float32)
    counts = _counts()
    in_maps = []
    for cid in range(NCORES):
        b_idx, hq = divmod(cid, 4)
        h0 = hq * HQ
        x_core = np.empty((NT, 128, 2, 2, HQ, WD), dtype=_NPF8)
        for i in range(NT):
            for v in range(2):
                # [C, HQ, WD] -> [kh, 128, HQ, WD] -> [128, kh, HQ, WD]
                blk = x_hilo[i][v][b_idx, :, h0 : h0 + HQ, :].reshape(2, 128, HQ, WD)
                x_core[i, :, v] = blk.transpose(1, 0, 2, 3)
        # bv[p, ver, mh, i*HQ + hl] = scale * b[mh*128+p] * count(12*(h0+hl) + i)
        # with scale = WS for the DVE table (ver 0), 1 for ACT (ver 1)
        i_idx = np.arange(NT)[:, None]
        hl_idx = np.arange(HQ)[None, :]
        cnt = counts[12 * (h0 + hl_idx) + i_idx].reshape(NT * HQ)  # [192]
        base = b.reshape(2, 128).T[:, None, :, None] * cnt[None, None, None, :]
        bv = (np.array([WS, 1.0], dtype=np.float32)[None, :, None, None]
              * base).astype(np.float32)  # [128, 2, 2, 192]
        in_maps.append({"x": x_core, "w": w_pack, "bv": bv})
    return in_maps


def gather_outputs(results: list[dict]) -> np.ndarray:
    out = np.empty((B, C, HOUT, WD), dtype=np.float32)
    for cid in range(NCORES):
        b_idx, hq = divmod(cid, 4)
        h0 = hq * HQ
        y = np.asarray(results[cid]["y"])  # [2, 128, HQ, NT, WD] bf16
        out[b_idx, :, 12 * h0 : 12 * h0 + NT * HQ, :] = (
            y.reshape(C, NT * HQ, WD).astype(np.float32)
        )
    return out


def kernel(**inputs) -> np.ndarray:
    nc = build_nc()
    in_maps = shard_inputs(inputs)
    res = run_bass_kernel_spmd(nc, in_maps, core_ids=list(range(NCORES)))
    return gather_outputs(res.results)
